# revision 2
# baseline (speedup 1.0000x reference)
"""BackgroundNoiseLayer kernel for 8 trn2 NeuronCores.

Math: out[0, t, n] = sum_k W[n, k] * rest[t, k], where W [60000, 100] is
scatter-added from COO (v1 block rows 0..49999, lm block rows 50000..59999)
and the output feature axis is the concat of the two blocks.

Strategy (per sharding hint): densify the tiny sparse matrix host-side
(240k nnz -> dense W, ~0.002% of the matmul FLOPs), shard the post-synaptic
feature axis across the 8 cores (7500 features each), and run a dense
[1000,101] @ [101,7500] matmul per core. rest is tiny and replicated. Each
core writes its own contiguous output slice; concat on host is the no-op
gather.

Precision scheme (gate is rel_err < 2e-2): the device emits a per-feature
scaled int8 stream. Host folds 127/s_n into W (s_n = 5 sigma of feature n,
computed exactly from the actual rest moments) and appends a constant-1
column to rest carrying -127*mu_n/s_n, so PSUM holds the centered, scaled
value in [-127,127]. The copy out of PSUM casts f32->int8; host decodes
q*(s/127)+mu in f32. Measured rel err ~8.7e-3.

v2 pipeline (from baseline trace analysis; baseline = 56.5us):
- Three stations: PE (bf16 matmuls, 512-col chunks into [128,1024] f32 PSUM
  tiles x4 = all 8 banks), PSUM evacuation (DVE CAST ~1.13us + ACT
  activation-copy ~1.03us per 1024-tile, both 1x mode - the saturated
  station ~32-34us), out-DMA.
- Baseline losses fixed here:
  * 12us tail: the single sync HWDGE queue sustains only ~188 GB/s
    aggregate < the ~218 GB/s copy production rate, so output backlogs and
    drains after compute ends. Fix: stageA halves ride sync (queue 1),
    stageB halves ride gpsimd SWDGE (queue 0) -> ~380 GB/s combined.
  * 4us startup: input DMA issue + DVE memset + 8 warmups serialized before
    first copy at t0+7.6us. Fix: memset on idle gpsimd, 5 warmups, inputs
    issued immediately (sync: rest0+w0, scalar: w1+rest1, gpsimd: w2-w7),
    first copy ~t0+4us.
  * unbalanced DVE/ACT split (32/32): ACT is ~10% faster per tile, so it
    takes 36 of 64 chunks including all eight 332-wide tails.
  * padding: contraction rows 112->101 (smaller w DMAs), output rows
    1024->1000 (last row block is 104 rows; 2.3% fewer HBM bytes).
- PE p-state: 5 warmup matmuls on a garbage-free scratch keep the HAM
  clock ungated until real work arrives; the steady pipeline keeps PE
  waits short so it stays at 2.4 GHz.
"""

import os

import numpy as np

B, T = 1, 1000
NBKG = 100
NV1, NLM = 50000, 10000
NPOST = NV1 + NLM          # 60000
NCORES = 8
SHARD = NPOST // NCORES    # 7500 real features per core

KP = 101                   # contraction dim (100 real + 1 bias col)
ROWS = 1000                # real time rows, no padding
TBLK = 128                 # rows per full block
NT = 8                     # row blocks: 7 x 128 + 1 x 104
LAST_ROWS = ROWS - 7 * TBLK  # 104
MMN = 512                  # matmul free dim cap = one fp32 PSUM bank
DCW = 1024                 # chunk width (2 PSUM banks)
DCHUNKS = [(i * DCW, DCW) for i in range(7)] + [(7 * DCW, SHARD - 7 * DCW)]
HSPLIT = 4 * DCW           # 4096: stageA | stageB split
ALPHA = 5.0                # int8 scale: s_n = ALPHA * sigma_n

_compiled = None


def _build_module():
    import concourse.bacc as bacc
    import concourse.mybir as mybir
    import concourse.tile as tile

    f32 = mybir.dt.float32
    i8 = mybir.dt.int8
    bf16 = mybir.dt.bfloat16
    nc = bacc.Bacc("TRN2", target_bir_lowering=False, debug=False)
    restT = nc.dram_tensor("restT", [KP, ROWS], bf16, kind="ExternalInput")
    wT = nc.dram_tensor("wT", [KP, SHARD], bf16, kind="ExternalInput")
    out = nc.dram_tensor("out", [ROWS, SHARD], i8, kind="ExternalOutput")

    with tile.TileContext(nc) as tc:
        with (
            tc.tile_pool(name="inp", bufs=1) as inp,
            tc.tile_pool(name="stage", bufs=3) as stagep,
            tc.tile_pool(name="psum", bufs=4, space="PSUM") as psump,
        ):
            # Warmup scratch memset on gpsimd (idle at t0; DVE must stay
            # free for its first CAST). 5 dummy matmuls warm the PE HAM
            # clock gate while the first input DMAs are in flight.
            scratch = inp.tile([KP, 640], bf16, tag="warm")
            nc.gpsimd.memset(scratch[:], 0.0)

            # inputs: first row-blocks' lhsT + first w chunk on the sync
            # HWDGE ring (fast first byte); w1 + the remaining rest rows on
            # the scalar HWDGE ring (2 issues, done before ACT's first
            # copy); w2..w7 on the gpsimd SWDGE ring in consumption order.
            rest0 = inp.tile([KP, 2 * TBLK], bf16, tag="rest0")
            w_sb = []
            for j, (off, w) in enumerate(DCHUNKS):
                w_sb.append(inp.tile([KP, w], bf16, tag=f"w{j}", name=f"w{j}"))
            rest1 = inp.tile([KP, ROWS - 2 * TBLK], bf16, tag="rest1")

            nc.sync.dma_start(rest0[:], restT[:, :2 * TBLK])
            nc.sync.dma_start(w_sb[0][:], wT[:, 0:DCW])
            nc.scalar.dma_start(w_sb[1][:], wT[:, DCW:2 * DCW])
            nc.scalar.dma_start(rest1[:], restT[:, 2 * TBLK:])
            for j in range(2, 8):
                off, w = DCHUNKS[j]
                nc.gpsimd.dma_start(w_sb[j][:], wT[:, off:off + w])

            for _ in range(5):
                psw = psump.tile([TBLK, DCW], f32, tag="ps")
                nc.tensor.matmul(psw[:, :MMN], scratch[:, :TBLK],
                                 scratch[:, TBLK:TBLK + MMN],
                                 start=True, stop=True)

            # Copy engine split (measured: DVE ~1.13us, ACT ~1.03us per
            # 1024-chunk; ACT gets all 332-tails + half the fulls).
            # Even blocks: DVE {0,2,4,6}; odd blocks: DVE {1,3,5}.
            for tb in range(NT):
                rows = TBLK if tb < 7 else LAST_ROWS
                r0 = tb * TBLK
                vector_chunks = {0, 2, 4, 6} if tb % 2 == 0 else {1, 3, 5}
                stageA = stagep.tile([TBLK, HSPLIT], i8, tag="stA",
                                     name=f"stA{tb}", bufs=3)
                stageB = stagep.tile([TBLK, SHARD - HSPLIT], i8, tag="stB",
                                     name=f"stB{tb}", bufs=3)
                if tb < 2:
                    lhsT = rest0[:, tb * TBLK:tb * TBLK + rows]
                else:
                    lhsT = rest1[:, (tb - 2) * TBLK:(tb - 2) * TBLK + rows]
                for j, (off, w) in enumerate(DCHUNKS):
                    ps = psump.tile([TBLK, DCW], f32, tag="ps")
                    for m in range((w + MMN - 1) // MMN):
                        n0 = m * MMN
                        n1 = min(w, n0 + MMN)
                        nc.tensor.matmul(
                            ps[:rows, n0:n1],
                            lhsT,
                            w_sb[j][:, n0:n1],
                            start=True,
                            stop=True,
                        )
                    copy = (nc.vector.tensor_copy if j in vector_chunks
                            else nc.scalar.copy)
                    if off < HSPLIT:
                        copy(stageA[:rows, off:off + w], ps[:rows, :w])
                    else:
                        copy(stageB[:rows, off - HSPLIT:off - HSPLIT + w],
                             ps[:rows, :w])
                    # stageA halves ride the sync HWDGE queue, stageB halves
                    # the gpsimd SWDGE queue: two independent DMA queues so
                    # the out stream drains at production rate. Last block
                    # goes out as quarters to shorten the final drain.
                    if j == 3:
                        if tb < 7:
                            nc.sync.dma_start(out[r0:r0 + rows, :HSPLIT],
                                              stageA[:rows, :])
                        else:
                            h = HSPLIT // 2
                            nc.sync.dma_start(out[r0:r0 + rows, :h],
                                              stageA[:rows, :h])
                            nc.sync.dma_start(out[r0:r0 + rows, h:HSPLIT],
                                              stageA[:rows, h:])
                    elif j == 7:
                        if tb < 7:
                            nc.gpsimd.dma_start(out[r0:r0 + rows, HSPLIT:],
                                                stageB[:rows, :])
                        else:
                            h = (SHARD - HSPLIT) // 2
                            nc.gpsimd.dma_start(
                                out[r0:r0 + rows, HSPLIT:HSPLIT + h],
                                stageB[:rows, :h])
                            nc.gpsimd.dma_start(
                                out[r0:r0 + rows, HSPLIT + h:],
                                stageB[:rows, h:])

    nc.compile()
    return nc


def _densify(v1_weights, v1_rows, v1_cols, lm_weights, lm_rows, lm_cols):
    rows = np.concatenate([
        np.asarray(v1_rows).astype(np.int64),
        np.asarray(lm_rows).astype(np.int64) + NV1,
    ])
    cols = np.concatenate([
        np.asarray(v1_cols).astype(np.int64),
        np.asarray(lm_cols).astype(np.int64),
    ])
    w = np.concatenate([
        np.asarray(v1_weights, dtype=np.float32),
        np.asarray(lm_weights, dtype=np.float32),
    ])
    W = np.bincount(rows * NBKG + cols, weights=w, minlength=NPOST * NBKG)
    return W.astype(np.float32).reshape(NPOST, NBKG)


def kernel(rest, v1_weights, v1_rows, v1_cols, lm_weights, lm_rows, lm_cols):
    import ml_dtypes

    from concourse.bass_utils import run_bass_kernel_spmd

    bf16 = ml_dtypes.bfloat16

    global _compiled
    if _compiled is None:
        _compiled = _build_module()

    W = _densify(v1_weights, v1_rows, v1_cols, lm_weights, lm_rows, lm_cols)
    rest32 = np.asarray(rest, np.float32)

    # per-feature affine int8 code: psum = 127*(out - mu)/s, decoded
    # host-side as q*(s/127) + mu. mu and sigma are exact moments of the
    # actual rest sample, so s = ALPHA*sigma covers the deviations.
    lam = rest32.mean(0)                       # [NBKG]
    var = ((rest32 - lam) ** 2).mean(0)        # [NBKG]
    mu = W @ lam                               # [NPOST]
    sig = np.sqrt(np.maximum((W * W) @ var, 1e-12))
    s = ALPHA * sig
    Wq = W * (127.0 / s)[:, None]              # [NPOST, NBKG]
    muq = -127.0 * mu / s                      # [NPOST]

    restT = np.zeros((KP, ROWS), bf16)
    restT[:NBKG, :] = rest32.astype(bf16).T
    restT[NBKG, :] = bf16(1.0)                 # bias column

    in_maps = []
    for c in range(NCORES):
        sl = slice(c * SHARD, (c + 1) * SHARD)
        wpad = np.zeros((KP, SHARD), bf16)
        wpad[:NBKG, :] = Wq[sl].T.astype(bf16)
        wpad[NBKG, :] = muq[sl].astype(bf16)
        in_maps.append({"restT": restT, "wT": wpad})

    trace = bool(int(os.environ.get("KERNEL_TRACE", "0")))
    if trace:
        _install_ntff_shim()
    res = run_bass_kernel_spmd(
        _compiled, in_maps, core_ids=list(range(NCORES)), trace=trace
    )
    kernel.last_results = res
    dec = [
        res.results[c]["out"].astype(np.float32)
        * (s[c * SHARD:(c + 1) * SHARD] / 127.0)[None, :]
        + mu[c * SHARD:(c + 1) * SHARD][None, :]
        for c in range(NCORES)
    ]
    full = np.concatenate(dec, axis=1)
    return full.reshape(B, T, NPOST)


def _install_ntff_shim():
    """The agent image's antenv lacks axon_hooks; register the NTFF profile
    hook by dlopening libaxon_pjrt.so directly (same path trn_boot uses)."""
    import sys
    import types

    if "antenv.axon_hooks" in sys.modules:
        return
    try:
        from trn_agent_boot.trn_boot import _ntff_profile_via_ctypes

        hook = _ntff_profile_via_ctypes("/opt/axon/libaxon_pjrt.so")
    except Exception:
        hook = None
    mod = types.ModuleType("antenv.axon_hooks")
    mod.get_axon_ntff_profile_hook = lambda: hook
    mod.set_axon_ntff_profile_hook = lambda h: None
    sys.modules["antenv.axon_hooks"] = mod


# revision 3
# speedup vs baseline: 1.2109x; 1.2109x over previous
"""BackgroundNoiseLayer kernel for 8 trn2 NeuronCores.

Math: out[0, t, n] = sum_k W[n, k] * rest[t, k], where W [60000, 100] is
scatter-added from COO (v1 block rows 0..49999, lm block rows 50000..59999)
and the output feature axis is the concat of the two blocks.

Strategy (per sharding hint): densify the tiny sparse matrix host-side
(240k nnz -> dense W, ~0.002% of the matmul FLOPs), shard the post-synaptic
feature axis across the 8 cores (7500 features each), and run a dense
[1000,101] @ [101,7500] matmul per core. rest is tiny and replicated. Each
core writes its own contiguous output slice; concat on host is the no-op
gather.

Precision scheme (gate is rel_err < 2e-2): the device emits a per-feature
scaled int8 stream. Host folds 127/s_n into W (s_n = 5 sigma of feature n,
computed exactly from the actual rest moments) and appends a constant-1
column to rest carrying -127*mu_n/s_n, so PSUM holds the centered, scaled
value in [-127,127]. The copy out of PSUM casts f32->int8; host decodes
q*(s/127)+mu in f32. Measured rel err ~8.7e-3.

v2 pipeline (from baseline trace analysis; baseline = 56.5us):
- Three stations: PE (bf16 matmuls, 512-col chunks into [128,1024] f32 PSUM
  tiles x4 = all 8 banks), PSUM evacuation (DVE CAST ~1.13us + ACT
  activation-copy ~1.03us per 1024-tile, both 1x mode - the saturated
  station ~32-34us), out-DMA.
- Baseline losses fixed here:
  * 12us tail: the single sync HWDGE queue sustains only ~188 GB/s
    aggregate < the ~218 GB/s copy production rate, so output backlogs and
    drains after compute ends. Fix: stageA halves ride sync (queue 1),
    stageB halves ride gpsimd SWDGE (queue 0) -> ~380 GB/s combined.
  * 4us startup: input DMA issue + DVE memset + 8 warmups serialized before
    first copy at t0+7.6us. Fix: memset on idle gpsimd, 5 warmups, inputs
    issued immediately (sync: rest0+w0, scalar: w1+rest1, gpsimd: w2-w7),
    first copy ~t0+4us.
  * unbalanced DVE/ACT split (32/32): ACT is ~10% faster per tile, so it
    takes 36 of 64 chunks including all eight 332-wide tails.
  * padding: contraction rows 112->101 (smaller w DMAs), output rows
    1024->1000 (last row block is 104 rows; 2.3% fewer HBM bytes).
- PE p-state: 5 warmup matmuls on a garbage-free scratch keep the HAM
  clock ungated until real work arrives; the steady pipeline keeps PE
  waits short so it stays at 2.4 GHz.
"""

import os

import numpy as np

B, T = 1, 1000
NBKG = 100
NV1, NLM = 50000, 10000
NPOST = NV1 + NLM          # 60000
NCORES = 8
SHARD = NPOST // NCORES    # 7500 real features per core

KP = 101                   # contraction dim (100 real + 1 bias col)
ROWS = 1000                # real time rows, no padding
TBLK = 128                 # rows per full block
NT = 8                     # row blocks: 7 x 128 + 1 x 104
LAST_ROWS = ROWS - 7 * TBLK  # 104
MMN = 512                  # matmul free dim cap = one fp32 PSUM bank
DCW = 1024                 # chunk width (2 PSUM banks)
DCHUNKS = [(i * DCW, DCW) for i in range(7)] + [(7 * DCW, SHARD - 7 * DCW)]
HSPLIT = 4 * DCW           # 4096: stageA | stageB split
ALPHA = 5.0                # int8 scale: s_n = ALPHA * sigma_n

_compiled = None


def _build_module():
    import concourse.bacc as bacc
    import concourse.mybir as mybir
    import concourse.tile as tile

    f32 = mybir.dt.float32
    i8 = mybir.dt.int8
    bf16 = mybir.dt.bfloat16
    nc = bacc.Bacc("TRN2", target_bir_lowering=False, debug=False)
    restT = nc.dram_tensor("restT", [KP, ROWS], bf16, kind="ExternalInput")
    wT = nc.dram_tensor("wT", [KP, SHARD], bf16, kind="ExternalInput")
    out = nc.dram_tensor("out", [ROWS, SHARD], i8, kind="ExternalOutput")

    with tile.TileContext(nc) as tc:
        with (
            tc.tile_pool(name="inp", bufs=1) as inp,
            tc.tile_pool(name="stage", bufs=3) as stagep,
            tc.tile_pool(name="psum", bufs=4, space="PSUM") as psump,
        ):
            # Warmup scratch memset on gpsimd (idle at t0; DVE must stay
            # free for its first CAST). 5 dummy matmuls warm the PE HAM
            # clock gate while the first input DMAs are in flight.
            scratch = inp.tile([KP, 640], bf16, tag="warm")
            nc.gpsimd.memset(scratch[:], 0.0)

            # inputs: first row-blocks' lhsT + first w chunk on the sync
            # HWDGE ring (fast first byte); w1 + the remaining rest rows on
            # the scalar HWDGE ring (2 issues, done before ACT's first
            # copy); w2..w7 on the gpsimd SWDGE ring in consumption order.
            rest0 = inp.tile([KP, 2 * TBLK], bf16, tag="rest0")
            w_sb = []
            for j, (off, w) in enumerate(DCHUNKS):
                w_sb.append(inp.tile([KP, w], bf16, tag=f"w{j}", name=f"w{j}"))
            rest1 = inp.tile([KP, ROWS - 2 * TBLK], bf16, tag="rest1")

            # NOTE: the scalar HWDGE queue is unusable here - its transfers
            # only progress in gaps of the (saturated) scalar engine, and a
            # scalar DMA_DIRECT2D issue costs ~2.8us. Everything rides sync
            # HWDGE + gpsimd SWDGE.
            nc.sync.dma_start(rest0[:], restT[:, :2 * TBLK])
            nc.sync.dma_start(w_sb[0][:], wT[:, 0:DCW])
            nc.sync.dma_start(w_sb[1][:], wT[:, DCW:2 * DCW])
            nc.gpsimd.dma_start(rest1[:], restT[:, 2 * TBLK:])
            for j in range(2, 8):
                off, w = DCHUNKS[j]
                nc.gpsimd.dma_start(w_sb[j][:], wT[:, off:off + w])

            for _ in range(5):
                psw = psump.tile([TBLK, DCW], f32, tag="ps")
                nc.tensor.matmul(psw[:, :MMN], scratch[:, :TBLK],
                                 scratch[:, TBLK:TBLK + MMN],
                                 start=True, stop=True)

            # Copy engine split (measured: DVE ~1.13us, ACT ~1.03us per
            # 1024-chunk; ACT gets all 332-tails + half the fulls).
            # Even blocks: DVE {0,2,4,6}; odd blocks: DVE {1,3,5}.
            for tb in range(NT):
                rows = TBLK if tb < 7 else LAST_ROWS
                r0 = tb * TBLK
                vector_chunks = {0, 2, 4, 6} if tb % 2 == 0 else {1, 3, 5}
                stageA = stagep.tile([TBLK, HSPLIT], i8, tag="stA",
                                     name=f"stA{tb}", bufs=3)
                stageB = stagep.tile([TBLK, SHARD - HSPLIT], i8, tag="stB",
                                     name=f"stB{tb}", bufs=3)
                if tb < 2:
                    lhsT = rest0[:, tb * TBLK:tb * TBLK + rows]
                else:
                    lhsT = rest1[:, (tb - 2) * TBLK:(tb - 2) * TBLK + rows]
                for j, (off, w) in enumerate(DCHUNKS):
                    ps = psump.tile([TBLK, DCW], f32, tag="ps")
                    for m in range((w + MMN - 1) // MMN):
                        n0 = m * MMN
                        n1 = min(w, n0 + MMN)
                        nc.tensor.matmul(
                            ps[:rows, n0:n1],
                            lhsT,
                            w_sb[j][:, n0:n1],
                            start=True,
                            stop=True,
                        )
                    copy = (nc.vector.tensor_copy if j in vector_chunks
                            else nc.scalar.copy)
                    if off < HSPLIT:
                        copy(stageA[:rows, off:off + w], ps[:rows, :w])
                    else:
                        copy(stageB[:rows, off - HSPLIT:off - HSPLIT + w],
                             ps[:rows, :w])
                    # stageA halves ride the sync HWDGE queue, stageB halves
                    # the gpsimd SWDGE queue: two independent DMA queues so
                    # the out stream drains at production rate. Last block
                    # goes out as quarters to shorten the final drain.
                    if j == 3:
                        if tb < 7:
                            nc.sync.dma_start(out[r0:r0 + rows, :HSPLIT],
                                              stageA[:rows, :])
                        else:
                            h = HSPLIT // 2
                            nc.sync.dma_start(out[r0:r0 + rows, :h],
                                              stageA[:rows, :h])
                            nc.sync.dma_start(out[r0:r0 + rows, h:HSPLIT],
                                              stageA[:rows, h:])
                    elif j == 7:
                        if tb < 7:
                            nc.gpsimd.dma_start(out[r0:r0 + rows, HSPLIT:],
                                                stageB[:rows, :])
                        else:
                            h = (SHARD - HSPLIT) // 2
                            nc.gpsimd.dma_start(
                                out[r0:r0 + rows, HSPLIT:HSPLIT + h],
                                stageB[:rows, :h])
                            nc.gpsimd.dma_start(
                                out[r0:r0 + rows, HSPLIT + h:],
                                stageB[:rows, h:])

    nc.compile()
    return nc


def _densify(v1_weights, v1_rows, v1_cols, lm_weights, lm_rows, lm_cols):
    rows = np.concatenate([
        np.asarray(v1_rows).astype(np.int64),
        np.asarray(lm_rows).astype(np.int64) + NV1,
    ])
    cols = np.concatenate([
        np.asarray(v1_cols).astype(np.int64),
        np.asarray(lm_cols).astype(np.int64),
    ])
    w = np.concatenate([
        np.asarray(v1_weights, dtype=np.float32),
        np.asarray(lm_weights, dtype=np.float32),
    ])
    W = np.bincount(rows * NBKG + cols, weights=w, minlength=NPOST * NBKG)
    return W.astype(np.float32).reshape(NPOST, NBKG)


def kernel(rest, v1_weights, v1_rows, v1_cols, lm_weights, lm_rows, lm_cols):
    import ml_dtypes

    from concourse.bass_utils import run_bass_kernel_spmd

    bf16 = ml_dtypes.bfloat16

    global _compiled
    if _compiled is None:
        _compiled = _build_module()

    W = _densify(v1_weights, v1_rows, v1_cols, lm_weights, lm_rows, lm_cols)
    rest32 = np.asarray(rest, np.float32)

    # per-feature affine int8 code: psum = 127*(out - mu)/s, decoded
    # host-side as q*(s/127) + mu. mu and sigma are exact moments of the
    # actual rest sample, so s = ALPHA*sigma covers the deviations.
    lam = rest32.mean(0)                       # [NBKG]
    var = ((rest32 - lam) ** 2).mean(0)        # [NBKG]
    mu = W @ lam                               # [NPOST]
    sig = np.sqrt(np.maximum((W * W) @ var, 1e-12))
    s = ALPHA * sig
    Wq = W * (127.0 / s)[:, None]              # [NPOST, NBKG]
    muq = -127.0 * mu / s                      # [NPOST]

    restT = np.zeros((KP, ROWS), bf16)
    restT[:NBKG, :] = rest32.astype(bf16).T
    restT[NBKG, :] = bf16(1.0)                 # bias column

    in_maps = []
    for c in range(NCORES):
        sl = slice(c * SHARD, (c + 1) * SHARD)
        wpad = np.zeros((KP, SHARD), bf16)
        wpad[:NBKG, :] = Wq[sl].T.astype(bf16)
        wpad[NBKG, :] = muq[sl].astype(bf16)
        in_maps.append({"restT": restT, "wT": wpad})

    trace = bool(int(os.environ.get("KERNEL_TRACE", "0")))
    if trace:
        _install_ntff_shim()
    res = run_bass_kernel_spmd(
        _compiled, in_maps, core_ids=list(range(NCORES)), trace=trace
    )
    kernel.last_results = res
    dec = [
        res.results[c]["out"].astype(np.float32)
        * (s[c * SHARD:(c + 1) * SHARD] / 127.0)[None, :]
        + mu[c * SHARD:(c + 1) * SHARD][None, :]
        for c in range(NCORES)
    ]
    full = np.concatenate(dec, axis=1)
    return full.reshape(B, T, NPOST)


def _install_ntff_shim():
    """The agent image's antenv lacks axon_hooks; register the NTFF profile
    hook by dlopening libaxon_pjrt.so directly (same path trn_boot uses)."""
    import sys
    import types

    if "antenv.axon_hooks" in sys.modules:
        return
    try:
        from trn_agent_boot.trn_boot import _ntff_profile_via_ctypes

        hook = _ntff_profile_via_ctypes("/opt/axon/libaxon_pjrt.so")
    except Exception:
        hook = None
    mod = types.ModuleType("antenv.axon_hooks")
    mod.get_axon_ntff_profile_hook = lambda: hook
    mod.set_axon_ntff_profile_hook = lambda h: None
    sys.modules["antenv.axon_hooks"] = mod
